# revision 9
# baseline (speedup 1.0000x reference)
"""Causal self-attention (B=2, T=2048, C=1024, H=16, D=64) on 8 trn2 NeuronCores.

Sharding: core c -> batch b = c // 4, head group g = c % 4 (heads 4g..4g+3).
Each core computes, for its batch and its 4 heads:
    qkT   = Wqk_local^T @ x_b^T          [512, 2048]   (q/k transposed layout)
    v     = x_b @ Wv_local               [2048, 256]   (natural layout)
    sT    = k q^T (per head)             [k, q] blocks; exp(s/8), causal mask
    pv    = (v|ones)^T @ exp(sT)         [128, q]: 64 attn rows + 64 denom rows
    y_par = attnT-contraction @ Wp_local [2048, 1024]
Host: y[b] = sum of the 4 partials + b_proj + (b_attn_v @ W_proj).

The host pre-transposes x (layout choice only - all FLOPs stay on device)
and column/row-shards the weights. b_attn(q,k) folded in via per-partition
activation bias; b_attn(v) and b_proj folded in on the host (exact since
softmax rows sum to 1).

Engine-lane constraint: DVE/ACT operands must share the partition window, so
attention rows live at partitions 0:64 for even heads and 64:128 for odd
heads (the v|ones weight column order flips per parity), and the reciprocal
row block is mirrored across the partition halves with a tiny SBUF->SBUF DMA.
"""

import os
import sys

import numpy as np

for _p in ("/opt/trn_rl_repo",):
    if _p not in sys.path:
        sys.path.insert(0, _p)

import concourse.bass as bass  # noqa: E402,F401
import concourse.mybir as mybir  # noqa: E402
import concourse.tile as tile  # noqa: E402
from concourse import bacc  # noqa: E402
from concourse.bass_utils import run_bass_kernel_spmd  # noqa: E402

B, T, C, H, D = 2, 2048, 1024, 16, 64
HL = 4          # heads per core
N_CORES = 8
QCH = 512       # q-chunk width (one PSUM bank of fp32)
NKT = T // 128  # 16 k-tiles per head
NQC = T // QCH  # 4 q-chunks

F32 = mybir.dt.float32

# matmul compute dtype: "f32" (2 cyc/row), "f32r" (1.5 cyc/row, relaxed mul)
MM_DT = os.environ.get("KMM_DT", "f32")

LAST_RESULT = None  # BassKernelResults of the most recent kernel() call


def _mm(ap):
    """View an fp32 AP with the matmul compute dtype (bit-identical layout)."""
    if MM_DT == "f32r":
        return ap.bitcast(mybir.dt.float32r)
    return ap


def _body(tc, debug_dumps=False):
    nc = tc.nc
    ACT = mybir.ActivationFunctionType

    xt = nc.dram_tensor("xt", [C, T], F32, kind="ExternalInput").ap()
    wqk = nc.dram_tensor("wqk", [C, 512], F32, kind="ExternalInput").ap()
    wv = nc.dram_tensor("wv", [C, 256], F32, kind="ExternalInput").ap()
    wp = nc.dram_tensor("wp", [256, C], F32, kind="ExternalInput").ap()
    bqk = nc.dram_tensor("bqk", [128, 4], F32, kind="ExternalInput").ap()
    tri = nc.dram_tensor("tri", [128, 128], F32, kind="ExternalInput").ap()
    y = nc.dram_tensor("y", [T, C], F32, kind="ExternalOutput").ap()

    # ---------------- persistent SBUF ----------------
    persist = tc.alloc_tile_pool(name="persist", bufs=1)
    qk_sb = persist.tile([128, 2, 2, T], F32, tag="qk")    # [p, hpair, q/k, t]
    v_sb = persist.tile([128, NKT, HL, 128], F32, tag="v")  # [p, ktile, h, d|ones]
    at_sb = persist.tile([128, 2, T], F32, tag="at")       # attnT [p, ctile, t]
    wp_sb = persist.tile([128, 2, C], F32, tag="wp")
    bqk_sb = persist.tile([128, 4], F32, tag="bqk")
    tri_sb = persist.tile([128, 128], F32, tag="tri")

    nc.sync.dma_start(out=wp_sb, in_=wp.rearrange("(c p) n -> p c n", p=128))
    nc.sync.dma_start(out=bqk_sb, in_=bqk)
    nc.sync.dma_start(out=tri_sb, in_=tri)
    # ones|v weight layout (all heads): ones cols 0:64 -> denominator rows 0:64
    # of the PV psum; v cols 64:128 -> attn rows 64:128.  (reciprocal_approx
    # is a custom DVE op that only works at partition base 0, so the denom
    # must always land in the low half.)
    nc.gpsimd.memset(v_sb[:, :, :, 0:64], 1.0)

    # ---------------- phases A-C: load x^T & W, qkv projections ----------------
    with tc.tile_pool(name="proj_in", bufs=1) as pin, \
         tc.tile_pool(name="ps_qk", bufs=2, space="PSUM") as ps_qk_pool, \
         tc.tile_pool(name="ps_v", bufs=2, space="PSUM") as ps_v_pool:
        xt_sb = pin.tile([128, 8, T], F32, tag="xt")
        wqk_sb = pin.tile([128, 8, 512], F32, tag="wqk")
        wv_sb = pin.tile([128, 8, 256], F32, tag="wv")
        xt_r = xt.rearrange("(c p) t -> p c t", p=128)
        wqk_r = wqk.rearrange("(c p) n -> p c n", p=128)
        wv_r = wv.rearrange("(c p) n -> p c n", p=128)
        for ck in range(8):
            nc.sync.dma_start(out=wqk_sb[:, ck, :], in_=wqk_r[:, ck, :])
            nc.sync.dma_start(out=wv_sb[:, ck, :], in_=wv_r[:, ck, :])
            nc.sync.dma_start(out=xt_sb[:, ck, :], in_=xt_r[:, ck, :])

        # qkT = Wqk^T @ x^T : psum[j, t] per (column slice s, t chunk)
        SLICE_MAP = {0: (0, 0), 1: (1, 0), 2: (0, 1), 3: (1, 1)}  # s -> (hp, qk)
        for s in range(4):
            hp, qk = SLICE_MAP[s]
            for tch in range(NQC):
                ps_qk = ps_qk_pool.tile([128, QCH], F32, tag="psqk")
                for ck in range(8):
                    nc.tensor.matmul(
                        ps_qk,
                        lhsT=_mm(wqk_sb[:, ck, s * 128:(s + 1) * 128]),
                        rhs=_mm(xt_sb[:, ck, tch * QCH:(tch + 1) * QCH]),
                        start=(ck == 0), stop=(ck == 7),
                    )
                nc.vector.tensor_scalar_add(
                    out=qk_sb[:, hp, qk, tch * QCH:(tch + 1) * QCH],
                    in0=ps_qk, scalar1=bqk_sb[:, s:s + 1],
                )

        # v = x @ Wv : natural layout, scattered into per-parity column slots
        for kt in range(NKT):
            ps_v = ps_v_pool.tile([128, 256], F32, tag="psv")
            for ck in range(8):
                nc.tensor.matmul(
                    ps_v,
                    lhsT=_mm(xt_sb[:, ck, kt * 128:(kt + 1) * 128]),
                    rhs=_mm(wv_sb[:, ck, :]),
                    start=(ck == 0), stop=(ck == 7),
                )
            nc.vector.tensor_copy(
                out=v_sb[:, kt, :, 64:128],
                in_=ps_v.rearrange("p (h d) -> p h d", h=HL))

    # ---------------- phase D: attention per head ----------------
    with tc.tile_pool(name="ps_s", bufs=3, space="PSUM") as pss_pool, \
         tc.tile_pool(name="ps_pv", bufs=1, space="PSUM") as pv_pool, \
         tc.tile_pool(name="st", bufs=4) as st_pool, \
         tc.tile_pool(name="rc", bufs=2) as rc_pool, \
         tc.tile_pool(name="atmp", bufs=2) as atmp_pool:
        for h in range(HL):
            hp, off = h // 2, 64 * (h % 2)
            pv = [pv_pool.tile([128, QCH], F32, tag=f"pv{j}", name=f"pv{j}")
                  for j in range(NQC)]
            for i in range(NKT):
                j0 = i // 4
                for j in range(j0, NQC):
                    lo = i * 128 - j * QCH if j == j0 else 0
                    ps_s = pss_pool.tile([128, QCH], F32, tag="pss")
                    nc.tensor.matmul(
                        ps_s[:, lo:],
                        lhsT=_mm(qk_sb[off:off + 64, hp, 1, i * 128:(i + 1) * 128]),
                        rhs=_mm(qk_sb[off:off + 64, hp, 0,
                                      j * QCH + lo:(j + 1) * QCH]),
                        start=True, stop=True,
                    )
                    st = st_pool.tile([128, QCH], F32, tag="st")
                    if j == j0:
                        if lo > 0:
                            nc.vector.memset(st[:, 0:lo], 0.0)
                        nc.scalar.activation(
                            out=st[:, lo:], in_=ps_s[:, lo:],
                            func=ACT.Exp, scale=0.125,
                        )
                        nc.vector.tensor_mul(
                            out=st[:, lo:lo + 128],
                            in0=st[:, lo:lo + 128], in1=tri_sb,
                        )
                    else:
                        nc.scalar.activation(
                            out=st, in_=ps_s, func=ACT.Exp, scale=0.125,
                        )
                    nc.tensor.matmul(
                        pv[j],
                        lhsT=_mm(v_sb[:, i, h, :]),
                        rhs=_mm(st),
                        start=(i == 0), stop=(i == 4 * j + 3),
                    )
            for j in range(NQC):
                rc = rc_pool.tile([128, QCH], F32, tag="rc", name="rc")
                nc.vector.reciprocal_approx_fast(
                    out=rc[0:64, :], in_=pv[j][0:64, :])
                # mirror the reciprocal rows into the attn partition half
                nc.sync.dma_start(out=rc[64:128, :], in_=rc[0:64, :])
                js = slice(j * QCH, (j + 1) * QCH)
                if off == 64:
                    nc.vector.tensor_mul(
                        out=at_sb[64:128, hp, js],
                        in0=pv[j][64:128, :], in1=rc[64:128, :],
                    )
                else:
                    atmp = atmp_pool.tile([128, QCH], F32, tag="atmp",
                                          name="atmp")
                    nc.vector.tensor_mul(
                        out=atmp[64:128, :],
                        in0=pv[j][64:128, :], in1=rc[64:128, :],
                    )
                    nc.sync.dma_start(
                        out=at_sb[0:64, hp, js], in_=atmp[64:128, :])

    # ---------------- phase E: output projection ----------------
    with tc.tile_pool(name="ps_y", bufs=2, space="PSUM") as psy_pool, \
         tc.tile_pool(name="yo", bufs=3) as y_pool:
        for tt in range(NKT):
            for n2 in range(2):
                ps_y = psy_pool.tile([128, QCH], F32, tag="psy")
                for ct in range(2):
                    nc.tensor.matmul(
                        ps_y,
                        lhsT=_mm(at_sb[:, ct, tt * 128:(tt + 1) * 128]),
                        rhs=_mm(wp_sb[:, ct, n2 * QCH:(n2 + 1) * QCH]),
                        start=(ct == 0), stop=(ct == 1),
                    )
                yt = y_pool.tile([128, QCH], F32, tag="yt")
                nc.scalar.activation(out=yt, in_=ps_y, func=ACT.Copy)
                nc.sync.dma_start(
                    out=y[tt * 128:(tt + 1) * 128, n2 * QCH:(n2 + 1) * QCH],
                    in_=yt,
                )

    if debug_dumps:
        qk_d = nc.dram_tensor("qk_d", [128, 2, 2, T], F32, kind="ExternalOutput").ap()
        v_d = nc.dram_tensor("v_d", [128, NKT, HL, 128], F32,
                             kind="ExternalOutput").ap()
        at_d = nc.dram_tensor("at_d", [128, 2, T], F32, kind="ExternalOutput").ap()
        nc.sync.dma_start(out=qk_d, in_=qk_sb)
        nc.sync.dma_start(out=v_d, in_=v_sb)
        nc.sync.dma_start(out=at_d, in_=at_sb)

    persist.release()


_PROGRAM = None


def build_program(debug_dumps=False):
    global _PROGRAM
    if _PROGRAM is None or debug_dumps:
        nc = bacc.Bacc("TRN2", debug=False, num_devices=N_CORES)
        with tile.TileContext(nc) as tc:
            _body(tc, debug_dumps=debug_dumps)
        nc.compile()
        if debug_dumps:
            return nc
        _PROGRAM = nc
    return _PROGRAM


def make_in_maps(x, W_attn, b_attn, W_proj):
    """Host-side shard: per-core input dict."""
    x = np.asarray(x, np.float32)
    W_attn = np.asarray(W_attn, np.float32)
    b_attn = np.asarray(b_attn, np.float32)
    W_proj = np.asarray(W_proj, np.float32)
    tri = np.triu(np.ones((128, 128), np.float32))  # tri[k, q] = k <= q
    in_maps = []
    for c in range(N_CORES):
        b, g = divmod(c, 4)
        xt = np.ascontiguousarray(x[b].T)  # [C, T]
        q0 = 256 * g
        cols = np.r_[q0:q0 + 256, C + q0:C + q0 + 256]  # q then k, heads 4g..4g+3
        wqk = np.ascontiguousarray(W_attn[:, cols])  # [C, 512] = [q01|q23|k01|k23]
        wv = np.ascontiguousarray(W_attn[:, 2 * C + q0:2 * C + q0 + 256])
        wp_l = np.ascontiguousarray(W_proj[q0:q0 + 256, :])
        bqk = np.ascontiguousarray(
            b_attn[cols].reshape(4, 128).T)  # [128, 4], col s = slice s bias
        in_maps.append({
            "xt": xt, "wqk": wqk, "wv": wv, "wp": wp_l,
            "bqk": bqk, "tri": tri,
        })
    return in_maps


def kernel(x, W_attn, b_attn, W_proj, b_proj):
    global LAST_RESULT
    W_attn = np.asarray(W_attn, np.float32)
    W_proj = np.asarray(W_proj, np.float32)
    b_attn = np.asarray(b_attn, np.float32)
    b_proj = np.asarray(b_proj, np.float32)

    nc = build_program()
    in_maps = make_in_maps(x, W_attn, b_attn, W_proj)
    res = run_bass_kernel_spmd(nc, in_maps, core_ids=list(range(N_CORES)))
    LAST_RESULT = res
    parts = [r["y"] for r in res.results]
    yb = [parts[0] + parts[1] + parts[2] + parts[3],
          parts[4] + parts[5] + parts[6] + parts[7]]
    out = np.stack(yb, axis=0)  # [B, T, C]
    # host-folded biases: b_proj, and the v-part of b_attn (softmax rows sum to 1)
    out += (b_proj + b_attn[2 * C:] @ W_proj)[None, None, :]
    return out.astype(np.float32)
